# revision 23
# baseline (speedup 1.0000x reference)
"""ComPoM sparse-attention kernel for 8 TRN2 NeuronCores (v3).

Math (per batch b), computed exactly (hardsigmoid kept exact):
    h  = (mask*xc) @ Wpo.T                 (N, DE)   [mask folded: a(0)=0]
    a  = lrelu(h, 0.01)                    (clips never fire for these stats)
    Sk = sum_n a^k  (k=1..3),  cnt = sum_n mask
    hm = (c0*S1 + c1*S2 + c2*S3) / cnt     (DE,)
    z  = xq @ Wse.T                        (T, DE)
    s  = min(relu(z/6 + bse/6 + .5), 1) * hm     [hm folded per-channel]
    out= s @ Wag.T                         (T, DIM)

Sharding over 8 cores: core c -> batch b = c//2, half j = c%2:
  stage 1 over N-half j (full DE) -> partial S; 24KB AllReduce(add) of S
  within the pair; stage 2 (z + final) over T-half j.

Schedule: xq prep (cast+PE transpose) is pipelined inside the stage-1
loop; all four z panels run before the first final matmul so the PE
never waits for the S-collective -> hm -> s-rescale chain. dtypes: h/z
matmuls fp8 e4m3 DoubleRow (2x PE rate, weights prescaled x16), final
matmul bf16, fp32 psum; outputs DMA straight from PSUM.
"""

import numpy as np
import ml_dtypes

import concourse.bacc as bacc
import concourse.bass as bass
import concourse.masks as masks
import concourse.mybir as mybir
import concourse.tile as tile
from concourse.bass_utils import run_bass_kernel_spmd

B, T, N, DIM = 4, 4096, 4096, 1024
EXPAND, DEGREE = 2, 3
DE = DIM * EXPAND
N_CORES = 8
TSH = T // 2       # stage-2 per-core row shard
NSH = N // 2       # stage-1 per-core row shard

P = 128
NCH = 512          # free-dim chunk (one fp32 PSUM bank)
NQ = DIM // 256    # 4 DoubleRow k-groups (256 contraction each)
NE = DE // P       # 16 e-tiles
NP = NSH // NCH    # 4 xc panels
NTP = TSH // NCH   # 4 xq panels
NSL = NCH // P     # 4 slabs per panel
NTB = NCH // P     # 4 t-blocks per panel
NDC = DIM // NCH   # 2 output d-chunks
WS = 16.0          # fp8 weight prescale (undone in ACT scale)

F32 = mybir.dt.float32
BF16 = mybir.dt.bfloat16
F8 = mybir.dt.float8e4
I32 = mybir.dt.int32
OP = mybir.AluOpType
AF = mybir.ActivationFunctionType
DR = mybir.MatmulPerfMode.DoubleRow

_CACHE = {}


def _build():
    nc = bacc.Bacc("TRN2", target_bir_lowering=False, debug=False,
                   enable_asserts=False, num_devices=N_CORES)

    xc_d = nc.dram_tensor("xc", [NSH, DIM], F32, kind="ExternalInput").ap()
    xq_d = nc.dram_tensor("xq", [TSH, DIM], F32, kind="ExternalInput").ap()
    # small tensors arrive host-packed in on-chip layout (partition-major)
    maskh_d = nc.dram_tensor("maskh", [P, NP * NSL], F32, kind="ExternalInput").ap()
    maskf_d = nc.dram_tensor("maskf", [P, N // P], F32, kind="ExternalInput").ap()
    wpo8_d = nc.dram_tensor("wpo8", [NQ, P, 2, DE], F8, kind="ExternalInput").ap()
    wse8_d = nc.dram_tensor("wse8", [NQ, P, 2, DE], F8, kind="ExternalInput").ap()
    wag_d = nc.dram_tensor("wag", [DE, DIM], BF16, kind="ExternalInput").ap()
    bseb_d = nc.dram_tensor("bseb", [P, NE], F32, kind="ExternalInput").ap()
    coeff_d = nc.dram_tensor("coeff", [P, NE, DEGREE], F32,
                             kind="ExternalInput").ap()
    out_d = nc.dram_tensor("out", [TSH, DIM], F32, kind="ExternalOutput").ap()

    with tile.TileContext(nc, trace_sim=False) as tc:
        with (
            tc.tile_pool(name="prep", bufs=1) as prep,
            tc.tile_pool(name="wts", bufs=1) as wts,
            tc.tile_pool(name="xqt", bufs=1) as xqt,
            tc.tile_pool(name="xs", bufs=2) as xs,
            tc.tile_pool(name="x8", bufs=2) as x8,
            tc.tile_pool(name="work", bufs=2) as work,
            tc.tile_pool(name="st", bufs=3) as st,
            tc.tile_pool(name="red", bufs=1) as red,
            tc.tile_pool(name="tps", bufs=2, space="PSUM") as tps,
            tc.tile_pool(name="ph", bufs=2, space="PSUM") as ph,
            tc.tile_pool(name="pz", bufs=2, space="PSUM") as pz,
            tc.tile_pool(name="po", bufs=2, space="PSUM") as po,
            tc.tile_pool(name="dram", bufs=1, space="DRAM") as dram,
        ):
            # ---- bulk loads first so stage 1 starts immediately ----------
            wpo8_sb = [wts.tile([P, 2, DE], F8, name=f"wpo8_{q}", tag=f"wpo8_{q}")
                       for q in range(NQ)]
            wse8_sb = [wts.tile([P, 2, DE], F8, name=f"wse8_{q}", tag=f"wse8_{q}")
                      for q in range(NQ)]
            wag_sb = [wts.tile([P, DIM], BF16, name=f"wag{e}", tag=f"wag{e}")
                      for e in range(NE)]
            nc.sync.dma_start(out=wpo8_sb[0][:], in_=wpo8_d[0])

            # small prep tensors ride the vector-engine DMA queue
            identf = prep.tile([P, P], F32, name="identf", tag="identf")
            masks.make_identity(nc, identf[:])
            identb = prep.tile([P, P], BF16, name="identb", tag="identb")
            nc.vector.tensor_copy(out=identb[:], in_=identf[:])
            mh_f = prep.tile([P, NP * NSL], F32, name="mh_f", tag="mh_f")
            nc.scalar.dma_start(out=mh_f[:], in_=maskh_d)
            rcnt_bc = prep.tile([P, 1], F32, name="rcnt_bc", tag="rcnt_bc")
            mf_f = prep.tile([P, N // P], F32, name="mf_f", tag="mf_f")
            nc.scalar.dma_start(out=mf_f[:], in_=maskf_d)
            cnt_p = prep.tile([P, 1], F32, name="cnt_p", tag="cnt_p")
            nc.vector.reduce_sum(out=cnt_p[:], in_=mf_f[:],
                                 axis=mybir.AxisListType.X)
            coeff_sb = prep.tile([P, NE, DEGREE], F32, name="coeff_sb",
                                 tag="coeff_sb")
            nc.scalar.dma_start(out=coeff_sb[:], in_=coeff_d)
            bseb_sb = prep.tile([P, NE], F32, name="bseb_sb", tag="bseb_sb")
            nc.scalar.dma_start(out=bseb_sb[:], in_=bseb_d)

            for q in range(1, NQ):
                nc.sync.dma_start(out=wpo8_sb[q][:], in_=wpo8_d[q])
            for q in range(NQ):
                nc.sync.dma_start(out=wse8_sb[q][:], in_=wse8_d[q])

            # xqT8[tp][q]: transposed fp8 xq panels, built during stage 1
            xqT8 = [[xqt.tile([P, 2, NCH], F8, name=f"xqT{tp}_{q}",
                              tag=f"xqT{tp}_{q}") for q in range(NQ)]
                    for tp in range(NTP)]

            def prep_xq_panel_xbar(pn, dst):
                """xq panel via XBAR DMA transpose: slab f32 -> bf16 cast ->
                dma_start_transpose (SBUF->SBUF) -> fp8 cast into DR tiles."""
                tbp = x8.tile([P, NSL, 2 * NQ, P], BF16, name="tbp", tag="tbp",
                              bufs=1)
                for s in range(NSL):
                    r0 = pn * NCH + s * P
                    sl = xs.tile([P, DIM], F32, name=f"sl{s}", tag=f"sl{s}")
                    nc.gpsimd.dma_start(out=sl[:], in_=xq_d[r0:r0 + P, :])
                    sm = x8.tile([P, DIM], BF16, name=f"sm{s}", tag=f"sm{s}")
                    nc.scalar.copy(out=sm[:], in_=sl[:])
                    nc.sync.dma_start_transpose(out=tbp[:, s, :, :], in_=sm[:])
                for q in range(NQ):
                    for i in range(2):
                        dd = 2 * q + i
                        nc.scalar.copy(
                            out=dst[q][:, i, :].rearrange("p (s c) -> p s c",
                                                          s=NSL),
                            in_=tbp[:, :, dd, :])

            def prep_panel(src_d, pn, dst, masked):
                """load panel pn of src, cast bf16 (mask-folded for xc),
                PE-transpose into fp8 DoubleRow tiles dst[q][P, 2, NCH]."""
                sms = []
                for s in range(NSL):
                    r0 = pn * NCH + s * P
                    sl = xs.tile([P, DIM], F32, name=f"sl{s}", tag=f"sl{s}")
                    nc.gpsimd.dma_start(out=sl[:], in_=src_d[r0:r0 + P, :])
                    sm = x8.tile([P, DIM], BF16, name=f"sm{s}", tag=f"sm{s}")
                    if masked:
                        mcol = mh_f[:, pn * NSL + s: pn * NSL + s + 1]
                        nc.vector.tensor_scalar(out=sm[:], in0=sl[:],
                                                scalar1=mcol, scalar2=None,
                                                op0=OP.mult)
                    else:
                        nc.scalar.copy(out=sm[:], in_=sl[:])
                    sms.append(sm)
                for q in range(NQ):
                    for i in range(2):
                        dd = 2 * q + i
                        pt = tps.tile([P, NCH], BF16, name="pt", tag="pt")
                        for s in range(NSL):
                            nc.tensor.transpose(
                                pt[:, s * P:(s + 1) * P],
                                sms[s][:, dd * P:(dd + 1) * P], identb[:])
                        if masked:
                            nc.vector.tensor_copy(out=dst[q][:, i, :], in_=pt[:])
                        else:
                            nc.scalar.copy(out=dst[q][:, i, :], in_=pt[:])

            # ---- stage 1 + pipelined xq prep -----------------------------
            Sraw = red.tile([P, DEGREE, NE, NP], F32, name="Sraw", tag="Sraw")
            xcT = [x8.tile([P, 2, NCH], F8, name=f"xcT{q}", tag=f"xcT{q}")
                   for q in range(NQ)]
            prep_panel(xc_d, 0, xcT, True)
            for pn in range(NP):
                xcT_cur = xcT
                for ei in range(NE):
                    ph_t = ph.tile([P, NCH], F32, name="h", tag="h")
                    for q in range(NQ):
                        nc.tensor.matmul(
                            ph_t[:], lhsT=wpo8_sb[q][:, :, ei * P:(ei + 1) * P],
                            rhs=xcT_cur[q][:], start=(q == 0),
                            stop=(q == NQ - 1), perf_mode=DR)
                    a = work.tile([P, NCH], BF16, name="a", tag="a")
                    nc.scalar.activation(
                        out=a[:], in_=ph_t[:], func=AF.Lrelu, alpha=0.01,
                        scale=1.0 / WS,
                        accum_out=Sraw[:, 0, ei, pn:pn + 1])
                    a2 = work.tile([P, NCH], BF16, name="a2", tag="a2")
                    nc.vector.scalar_tensor_tensor(
                        out=a2[:], in0=a[:], scalar=1.0, in1=a[:],
                        op0=OP.mult, op1=OP.mult,
                        accum_out=Sraw[:, 1, ei, pn:pn + 1])
                    nc.vector.scalar_tensor_tensor(
                        out=a2[:], in0=a2[:], scalar=1.0, in1=a[:],
                        op0=OP.mult, op1=OP.mult,
                        accum_out=Sraw[:, 2, ei, pn:pn + 1])
                # prefetch next xc panel + stage-2 input prep behind matmuls
                if pn + 1 < NP:
                    xcT = [x8.tile([P, 2, NCH], F8, name=f"xcT{q}",
                                   tag=f"xcT{q}") for q in range(NQ)]
                    prep_panel(xc_d, pn + 1, xcT, True)
                prep_xq_panel_xbar(pn, xqT8[pn])
                for e in (4 * pn, 4 * pn + 1, 4 * pn + 2, 4 * pn + 3):
                    nc.sync.dma_start(out=wag_sb[e][:],
                                      in_=wag_d[e * P:(e + 1) * P, :])

            # ---- S AllReduce within the batch pair + hm ------------------
            cnt_all = prep.tile([P, 1], F32, name="cnt_all", tag="cnt_all")
            nc.gpsimd.partition_all_reduce(
                cnt_all[:], cnt_p[:], channels=P,
                reduce_op=bass.bass_isa.ReduceOp.add)
            nc.vector.reciprocal(out=rcnt_bc[:], in_=cnt_all[:])
            Scat = red.tile([P, DEGREE * NE], F32, name="Scat", tag="Scat")
            for k in range(DEGREE):
                nc.vector.reduce_sum(
                    out=Scat[:, k * NE:(k + 1) * NE],
                    in_=Sraw[:, k, :, :], axis=mybir.AxisListType.X)
            s_dr = dram.tile([P * DEGREE * NE], F32, name="s_dr", tag="s_dr")
            sall_dr = dram.tile([P * DEGREE * NE], F32, name="sall_dr",
                                tag="sall_dr")
            nc.sync.dma_start(out=s_dr.rearrange("(p a) -> p a", p=P),
                              in_=Scat[:])
            nc.gpsimd.collective_compute(
                "AllReduce", OP.add,
                replica_groups=[[0, 1], [2, 3], [4, 5], [6, 7]],
                ins=[s_dr.opt()], outs=[sall_dr.opt()])
            Scall = red.tile([P, DEGREE * NE], F32, name="Scall", tag="Scall")
            nc.sync.dma_start(out=Scall[:],
                              in_=sall_dr.rearrange("(p a) -> p a", p=P))

            hm_sb = prep.tile([P, NE], F32, name="hm_sb", tag="hm_sb")
            u1 = red.tile([P, NE], F32, name="u1", tag="u1")
            u2 = red.tile([P, NE], F32, name="u2", tag="u2")
            u3 = red.tile([P, NE], F32, name="u3", tag="u3")
            nc.vector.tensor_tensor(out=u1[:], in0=Scall[:, 0:NE],
                                    in1=coeff_sb[:, :, 0], op=OP.mult)
            nc.vector.tensor_tensor(out=u2[:], in0=Scall[:, NE:2 * NE],
                                    in1=coeff_sb[:, :, 1], op=OP.mult)
            nc.vector.tensor_tensor(out=u3[:], in0=Scall[:, 2 * NE:3 * NE],
                                    in1=coeff_sb[:, :, 2], op=OP.mult)
            nc.vector.tensor_tensor(out=u1[:], in0=u1[:], in1=u2[:], op=OP.add)
            nc.vector.tensor_tensor(out=u1[:], in0=u1[:], in1=u3[:], op=OP.add)
            nc.vector.tensor_scalar(out=hm_sb[:], in0=u1[:],
                                    scalar1=rcnt_bc[:, 0:1], scalar2=None,
                                    op0=OP.mult)

            # ---- stage 2: all z panels first, then finals ----------------
            sT = [[None] * NE for _ in range(NTP)]

            def z_panel(tp):
                for ei in range(NE):
                    pz_t = pz.tile([P, NCH], F32, name="z", tag="z")
                    for q in range(NQ):
                        nc.tensor.matmul(
                            pz_t[:], lhsT=wse8_sb[q][:, :, ei * P:(ei + 1) * P],
                            rhs=xqT8[tp][q][:], start=(q == 0),
                            stop=(q == NQ - 1), perf_mode=DR)
                    sT_t = st.tile([P, NCH], BF16, name=f"sT{ei}", tag=f"sT{ei}")
                    nc.scalar.activation(out=sT_t[:], in_=pz_t[:], func=AF.Relu,
                                         bias=bseb_sb[:, ei:ei + 1],
                                         scale=1.0 / (WS * 6.0))
                    nc.vector.tensor_scalar(out=sT_t[:], in0=sT_t[:],
                                            scalar1=1.0,
                                            scalar2=hm_sb[:, ei:ei + 1],
                                            op0=OP.min, op1=OP.mult)
                    sT[tp][ei] = sT_t

            def final_panel(tp):
                for tb in range(NTB):
                    r0 = tp * NCH + tb * P
                    ob = work.tile([P, DIM], F32, name="ob", tag="ob")
                    for dc in range(NDC):
                        po_t = po.tile([P, NCH], F32, name="o", tag="o")
                        for ei in range(NE):
                            nc.tensor.matmul(
                                po_t[:], lhsT=sT[tp][ei][:, tb * P:(tb + 1) * P],
                                rhs=wag_sb[ei][:, dc * NCH:(dc + 1) * NCH],
                                start=(ei == 0), stop=(ei == NE - 1))
                        nc.scalar.copy(
                            out=ob[:, dc * NCH:(dc + 1) * NCH], in_=po_t[:])
                    nc.sync.dma_start(out=out_d[r0:r0 + P, :], in_=ob[:])

            # z0,z1,z2 run while the S collective + hm complete; finals
            # then never stall on the s-rescale chain.
            z_panel(0)
            z_panel(1)
            z_panel(2)
            final_panel(0)
            z_panel(3)
            final_panel(1)
            final_panel(2)
            final_panel(3)

    nc.compile()
    return nc


def _get_nc():
    if "nc" not in _CACHE:
        _CACHE["nc"] = _build()
    return _CACHE["nc"]


def _prep_weights(Wpo, Wse, bse, coeff, Wag):
    f8 = ml_dtypes.float8_e4m3

    def pack8(W):
        WT = np.ascontiguousarray(np.asarray(W, np.float32).T * WS)
        return np.ascontiguousarray(
            WT.reshape(NQ, 2, P, DE).transpose(0, 2, 1, 3)).astype(f8)

    wpo8 = pack8(Wpo)
    wse8 = pack8(Wse)
    wag = np.ascontiguousarray(
        np.asarray(Wag, np.float32).T).astype(ml_dtypes.bfloat16)
    bseb = np.ascontiguousarray(
        (np.asarray(bse, np.float32) / 6.0 + 0.5).reshape(NE, P).T)
    coeffp = np.ascontiguousarray(
        np.asarray(coeff, np.float32).reshape(NE, P, DEGREE).transpose(1, 0, 2))
    return wpo8, wse8, wag, bseb, coeffp


def kernel(xq, xc, mask, Wpo, Wse, bse, coeff, Wag, _trace=False):
    nc = _get_nc()
    xq = np.ascontiguousarray(xq, np.float32)
    xc = np.ascontiguousarray(xc, np.float32)
    mask = np.ascontiguousarray(mask, np.int32)
    wpo8, wse8, wag, bseb, coeffp = _prep_weights(Wpo, Wse, bse, coeff, Wag)
    maskf = mask.astype(np.float32)
    in_maps = []
    for c in range(N_CORES):
        b, j = c // 2, c % 2
        mh = np.ascontiguousarray(
            maskf[b, j * NSH:(j + 1) * NSH].reshape(NP * NSL, P).T)
        mf = np.ascontiguousarray(maskf[b].reshape(N // P, P).T)
        in_maps.append({
            "xc": np.ascontiguousarray(xc[b, j * NSH:(j + 1) * NSH]),
            "xq": np.ascontiguousarray(xq[b, j * TSH:(j + 1) * TSH]),
            "maskh": mh,
            "maskf": mf,
            "wpo8": wpo8,
            "wse8": wse8,
            "wag": wag,
            "bseb": bseb,
            "coeff": coeffp,
        })
    res = run_bass_kernel_spmd(nc, in_maps, list(range(N_CORES)), trace=_trace)
    out = np.empty((B, T, DIM), np.float32)
    for c in range(N_CORES):
        b, j = c // 2, c % 2
        out[b, j * TSH:(j + 1) * TSH] = res.results[c]["out"]
    if _trace:
        _CACHE["last_result"] = res
    return out


# revision 24
# speedup vs baseline: 1.3659x; 1.3659x over previous
"""ComPoM sparse-attention kernel for 8 TRN2 NeuronCores (v3).

Math (per batch b), computed exactly (hardsigmoid kept exact):
    h  = (mask*xc) @ Wpo.T                 (N, DE)   [mask folded: a(0)=0]
    a  = lrelu(h, 0.01)                    (clips never fire for these stats)
    Sk = sum_n a^k  (k=1..3),  cnt = sum_n mask
    hm = (c0*S1 + c1*S2 + c2*S3) / cnt     (DE,)
    z  = xq @ Wse.T                        (T, DE)
    s  = min(relu(z/6 + bse/6 + .5), 1) * hm     [hm folded per-channel]
    out= s @ Wag.T                         (T, DIM)

Sharding over 8 cores: core c -> batch b = c//2, half j = c%2:
  stage 1 over N-half j (full DE) -> partial S; 24KB AllReduce(add) of S
  within the pair; stage 2 (z + final) over T-half j.

Schedule: xq prep (cast+PE transpose) is pipelined inside the stage-1
loop; all four z panels run before the first final matmul so the PE
never waits for the S-collective -> hm -> s-rescale chain. dtypes: h/z
matmuls fp8 e4m3 DoubleRow (2x PE rate, weights prescaled x16), final
matmul bf16, fp32 psum; outputs DMA straight from PSUM.
"""

import numpy as np
import ml_dtypes

import concourse.bacc as bacc
import concourse.bass as bass
import concourse.masks as masks
import concourse.mybir as mybir
import concourse.tile as tile
from concourse.bass_utils import run_bass_kernel_spmd

B, T, N, DIM = 4, 4096, 4096, 1024
EXPAND, DEGREE = 2, 3
DE = DIM * EXPAND
N_CORES = 8
TSH = T // 2       # stage-2 per-core row shard
NSH = N // 2       # stage-1 per-core row shard

P = 128
NCH = 512          # free-dim chunk (one fp32 PSUM bank)
NQ = DIM // 256    # 4 DoubleRow k-groups (256 contraction each)
NE = DE // P       # 16 e-tiles
NP = NSH // NCH    # 4 xc panels
NTP = TSH // NCH   # 4 xq panels
NSL = NCH // P     # 4 slabs per panel
NTB = NCH // P     # 4 t-blocks per panel
NDC = DIM // NCH   # 2 output d-chunks
WS = 16.0          # fp8 weight prescale (undone in ACT scale)

F32 = mybir.dt.float32
BF16 = mybir.dt.bfloat16
F8 = mybir.dt.float8e4
I32 = mybir.dt.int32
OP = mybir.AluOpType
AF = mybir.ActivationFunctionType
DR = mybir.MatmulPerfMode.DoubleRow

_CACHE = {}


def _build():
    nc = bacc.Bacc("TRN2", target_bir_lowering=False, debug=False,
                   enable_asserts=False, num_devices=N_CORES)

    xc_d = nc.dram_tensor("xc", [NSH, DIM], F32, kind="ExternalInput").ap()
    xq_d = nc.dram_tensor("xq", [TSH, DIM], F32, kind="ExternalInput").ap()
    # small tensors arrive host-packed in on-chip layout (partition-major)
    maskh_d = nc.dram_tensor("maskh", [P, NP * NSL], F32, kind="ExternalInput").ap()
    maskf_d = nc.dram_tensor("maskf", [P, N // P], F32, kind="ExternalInput").ap()
    wpo8_d = nc.dram_tensor("wpo8", [NQ, P, 2, DE], F8, kind="ExternalInput").ap()
    wse8_d = nc.dram_tensor("wse8", [NQ, P, 2, DE], F8, kind="ExternalInput").ap()
    wag_d = nc.dram_tensor("wag", [DE, DIM], BF16, kind="ExternalInput").ap()
    bseb_d = nc.dram_tensor("bseb", [P, NE], F32, kind="ExternalInput").ap()
    coeff_d = nc.dram_tensor("coeff", [P, NE, DEGREE], F32,
                             kind="ExternalInput").ap()
    out_d = nc.dram_tensor("out", [TSH, DIM], F32, kind="ExternalOutput").ap()

    with tile.TileContext(nc, trace_sim=False) as tc:
        with (
            tc.tile_pool(name="prep", bufs=1) as prep,
            tc.tile_pool(name="wts", bufs=1) as wts,
            tc.tile_pool(name="xqt", bufs=1) as xqt,
            tc.tile_pool(name="xs", bufs=2) as xs,
            tc.tile_pool(name="x8", bufs=2) as x8,
            tc.tile_pool(name="work", bufs=2) as work,
            tc.tile_pool(name="st", bufs=3) as st,
            tc.tile_pool(name="red", bufs=1) as red,
            tc.tile_pool(name="tps", bufs=2, space="PSUM") as tps,
            tc.tile_pool(name="ph", bufs=2, space="PSUM") as ph,
            tc.tile_pool(name="pz", bufs=2, space="PSUM") as pz,
            tc.tile_pool(name="po", bufs=2, space="PSUM") as po,
            tc.tile_pool(name="dram", bufs=1, space="DRAM") as dram,
        ):
            # ---- bulk loads first so stage 1 starts immediately ----------
            wpo8_sb = [wts.tile([P, 2, DE], F8, name=f"wpo8_{q}", tag=f"wpo8_{q}")
                       for q in range(NQ)]
            wse8_sb = [wts.tile([P, 2, DE], F8, name=f"wse8_{q}", tag=f"wse8_{q}")
                      for q in range(NQ)]
            wag_sb = [wts.tile([P, DIM], BF16, name=f"wag{e}", tag=f"wag{e}")
                      for e in range(NE)]
            nc.sync.dma_start(out=wpo8_sb[0][:], in_=wpo8_d[0])

            # small prep tensors ride the vector-engine DMA queue
            identf = prep.tile([P, P], F32, name="identf", tag="identf")
            masks.make_identity(nc, identf[:])
            identb = prep.tile([P, P], BF16, name="identb", tag="identb")
            nc.vector.tensor_copy(out=identb[:], in_=identf[:])
            mh_f = prep.tile([P, NP * NSL], F32, name="mh_f", tag="mh_f")
            nc.scalar.dma_start(out=mh_f[:], in_=maskh_d)
            rcnt_bc = prep.tile([P, 1], F32, name="rcnt_bc", tag="rcnt_bc")
            mf_f = prep.tile([P, N // P], F32, name="mf_f", tag="mf_f")
            nc.scalar.dma_start(out=mf_f[:], in_=maskf_d)
            cnt_p = prep.tile([P, 1], F32, name="cnt_p", tag="cnt_p")
            nc.vector.reduce_sum(out=cnt_p[:], in_=mf_f[:],
                                 axis=mybir.AxisListType.X)
            coeff_sb = prep.tile([P, NE, DEGREE], F32, name="coeff_sb",
                                 tag="coeff_sb")
            nc.scalar.dma_start(out=coeff_sb[:], in_=coeff_d)
            bseb_sb = prep.tile([P, NE], F32, name="bseb_sb", tag="bseb_sb")
            nc.scalar.dma_start(out=bseb_sb[:], in_=bseb_d)

            for q in range(1, NQ):
                nc.sync.dma_start(out=wpo8_sb[q][:], in_=wpo8_d[q])
            for q in range(NQ):
                nc.sync.dma_start(out=wse8_sb[q][:], in_=wse8_d[q])

            # xqT8[tp][q]: transposed fp8 xq panels, built during stage 1
            xqT8 = [[xqt.tile([P, 2, NCH], F8, name=f"xqT{tp}_{q}",
                              tag=f"xqT{tp}_{q}") for q in range(NQ)]
                    for tp in range(NTP)]

            def prep_xq_panel_xbar(pn, dst):
                """xq panel via XBAR DMA transpose: slab f32 -> bf16 cast ->
                dma_start_transpose (SBUF->SBUF) -> fp8 cast into DR tiles."""
                tbp = x8.tile([P, NSL, 2 * NQ, P], BF16, name="tbp", tag="tbp",
                              bufs=1)
                for s in range(NSL):
                    r0 = pn * NCH + s * P
                    sl = xs.tile([P, DIM], F32, name=f"sl{s}", tag=f"sl{s}")
                    nc.gpsimd.dma_start(out=sl[:], in_=xq_d[r0:r0 + P, :])
                    sm = x8.tile([P, DIM], BF16, name=f"sm{s}", tag=f"sm{s}")
                    nc.scalar.copy(out=sm[:], in_=sl[:])
                    nc.sync.dma_start_transpose(out=tbp[:, s, :, :], in_=sm[:])
                for q in range(NQ):
                    for i in range(2):
                        dd = 2 * q + i
                        nc.scalar.copy(
                            out=dst[q][:, i, :].rearrange("p (s c) -> p s c",
                                                          s=NSL),
                            in_=tbp[:, :, dd, :])

            def prep_panel(src_d, pn, dst, masked):
                """load panel pn of src, cast bf16 (mask-folded for xc),
                PE-transpose into fp8 DoubleRow tiles dst[q][P, 2, NCH]."""
                sms = []
                for s in range(NSL):
                    r0 = pn * NCH + s * P
                    sl = xs.tile([P, DIM], F32, name=f"sl{s}", tag=f"sl{s}")
                    nc.gpsimd.dma_start(out=sl[:], in_=src_d[r0:r0 + P, :])
                    sm = x8.tile([P, DIM], BF16, name=f"sm{s}", tag=f"sm{s}")
                    if masked:
                        mcol = mh_f[:, pn * NSL + s: pn * NSL + s + 1]
                        nc.vector.tensor_scalar(out=sm[:], in0=sl[:],
                                                scalar1=mcol, scalar2=None,
                                                op0=OP.mult)
                    else:
                        nc.scalar.copy(out=sm[:], in_=sl[:])
                    sms.append(sm)
                for q in range(NQ):
                    for i in range(2):
                        dd = 2 * q + i
                        pt = tps.tile([P, NCH], BF16, name="pt", tag="pt")
                        for s in range(NSL):
                            nc.tensor.transpose(
                                pt[:, s * P:(s + 1) * P],
                                sms[s][:, dd * P:(dd + 1) * P], identb[:])
                        if masked:
                            nc.vector.tensor_copy(out=dst[q][:, i, :], in_=pt[:])
                        else:
                            nc.scalar.copy(out=dst[q][:, i, :], in_=pt[:])

            # ---- stage 1 + pipelined xq prep -----------------------------
            Sraw = red.tile([P, DEGREE, NE, NP], F32, name="Sraw", tag="Sraw")
            xcT = [x8.tile([P, 2, NCH], F8, name=f"xcT{q}", tag=f"xcT{q}")
                   for q in range(NQ)]
            prep_panel(xc_d, 0, xcT, True)
            for pn in range(NP):
                xcT_cur = xcT
                for ei in range(NE):
                    ph_t = ph.tile([P, NCH], F32, name="h", tag="h")
                    for q in range(NQ):
                        nc.tensor.matmul(
                            ph_t[:], lhsT=wpo8_sb[q][:, :, ei * P:(ei + 1) * P],
                            rhs=xcT_cur[q][:], start=(q == 0),
                            stop=(q == NQ - 1), perf_mode=DR)
                    a = work.tile([P, NCH], BF16, name="a", tag="a")
                    nc.scalar.activation(
                        out=a[:], in_=ph_t[:], func=AF.Lrelu, alpha=0.01,
                        scale=1.0 / WS,
                        accum_out=Sraw[:, 0, ei, pn:pn + 1])
                    a2 = work.tile([P, NCH], BF16, name="a2", tag="a2")
                    nc.vector.scalar_tensor_tensor(
                        out=a2[:], in0=a[:], scalar=1.0, in1=a[:],
                        op0=OP.mult, op1=OP.mult,
                        accum_out=Sraw[:, 1, ei, pn:pn + 1])
                    nc.vector.scalar_tensor_tensor(
                        out=a2[:], in0=a2[:], scalar=1.0, in1=a[:],
                        op0=OP.mult, op1=OP.mult,
                        accum_out=Sraw[:, 2, ei, pn:pn + 1])
                # prefetch next xc panel + stage-2 input prep behind matmuls
                if pn + 1 < NP:
                    xcT = [x8.tile([P, 2, NCH], F8, name=f"xcT{q}",
                                   tag=f"xcT{q}") for q in range(NQ)]
                    prep_panel(xc_d, pn + 1, xcT, True)
                prep_panel(xq_d, pn, xqT8[pn], False)
                for e in (4 * pn, 4 * pn + 1, 4 * pn + 2, 4 * pn + 3):
                    nc.sync.dma_start(out=wag_sb[e][:],
                                      in_=wag_d[e * P:(e + 1) * P, :])

            # ---- S AllReduce within the batch pair + hm ------------------
            cnt_all = prep.tile([P, 1], F32, name="cnt_all", tag="cnt_all")
            nc.gpsimd.partition_all_reduce(
                cnt_all[:], cnt_p[:], channels=P,
                reduce_op=bass.bass_isa.ReduceOp.add)
            nc.vector.reciprocal(out=rcnt_bc[:], in_=cnt_all[:])
            Scat = red.tile([P, DEGREE * NE], F32, name="Scat", tag="Scat")
            for k in range(DEGREE):
                nc.vector.reduce_sum(
                    out=Scat[:, k * NE:(k + 1) * NE],
                    in_=Sraw[:, k, :, :], axis=mybir.AxisListType.X)
            s_dr = dram.tile([P * DEGREE * NE], F32, name="s_dr", tag="s_dr")
            sall_dr = dram.tile([P * DEGREE * NE], F32, name="sall_dr",
                                tag="sall_dr")
            nc.sync.dma_start(out=s_dr.rearrange("(p a) -> p a", p=P),
                              in_=Scat[:])
            nc.gpsimd.collective_compute(
                "AllReduce", OP.add,
                replica_groups=[[0, 1], [2, 3], [4, 5], [6, 7]],
                ins=[s_dr.opt()], outs=[sall_dr.opt()])
            Scall = red.tile([P, DEGREE * NE], F32, name="Scall", tag="Scall")
            nc.sync.dma_start(out=Scall[:],
                              in_=sall_dr.rearrange("(p a) -> p a", p=P))

            hm_sb = prep.tile([P, NE], F32, name="hm_sb", tag="hm_sb")
            u1 = red.tile([P, NE], F32, name="u1", tag="u1")
            u2 = red.tile([P, NE], F32, name="u2", tag="u2")
            u3 = red.tile([P, NE], F32, name="u3", tag="u3")
            nc.vector.tensor_tensor(out=u1[:], in0=Scall[:, 0:NE],
                                    in1=coeff_sb[:, :, 0], op=OP.mult)
            nc.vector.tensor_tensor(out=u2[:], in0=Scall[:, NE:2 * NE],
                                    in1=coeff_sb[:, :, 1], op=OP.mult)
            nc.vector.tensor_tensor(out=u3[:], in0=Scall[:, 2 * NE:3 * NE],
                                    in1=coeff_sb[:, :, 2], op=OP.mult)
            nc.vector.tensor_tensor(out=u1[:], in0=u1[:], in1=u2[:], op=OP.add)
            nc.vector.tensor_tensor(out=u1[:], in0=u1[:], in1=u3[:], op=OP.add)
            nc.vector.tensor_scalar(out=hm_sb[:], in0=u1[:],
                                    scalar1=rcnt_bc[:, 0:1], scalar2=None,
                                    op0=OP.mult)

            # ---- stage 2: all z panels first, then finals ----------------
            sT = [[None] * NE for _ in range(NTP)]

            def z_panel(tp):
                for ei in range(NE):
                    pz_t = pz.tile([P, NCH], F32, name="z", tag="z")
                    for q in range(NQ):
                        nc.tensor.matmul(
                            pz_t[:], lhsT=wse8_sb[q][:, :, ei * P:(ei + 1) * P],
                            rhs=xqT8[tp][q][:], start=(q == 0),
                            stop=(q == NQ - 1), perf_mode=DR)
                    sT_t = st.tile([P, NCH], BF16, name=f"sT{ei}", tag=f"sT{ei}")
                    nc.scalar.activation(out=sT_t[:], in_=pz_t[:], func=AF.Relu,
                                         bias=bseb_sb[:, ei:ei + 1],
                                         scale=1.0 / (WS * 6.0))
                    nc.vector.tensor_scalar(out=sT_t[:], in0=sT_t[:],
                                            scalar1=1.0,
                                            scalar2=hm_sb[:, ei:ei + 1],
                                            op0=OP.min, op1=OP.mult)
                    sT[tp][ei] = sT_t

            def final_panel(tp):
                for tb in range(NTB):
                    r0 = tp * NCH + tb * P
                    ob = work.tile([P, DIM], F32, name="ob", tag="ob")
                    for dc in range(NDC):
                        po_t = po.tile([P, NCH], F32, name="o", tag="o")
                        for ei in range(NE):
                            nc.tensor.matmul(
                                po_t[:], lhsT=sT[tp][ei][:, tb * P:(tb + 1) * P],
                                rhs=wag_sb[ei][:, dc * NCH:(dc + 1) * NCH],
                                start=(ei == 0), stop=(ei == NE - 1))
                        nc.scalar.copy(
                            out=ob[:, dc * NCH:(dc + 1) * NCH], in_=po_t[:])
                    nc.sync.dma_start(out=out_d[r0:r0 + P, :], in_=ob[:])

            # z0,z1,z2 run while the S collective + hm complete; finals
            # then never stall on the s-rescale chain.
            z_panel(0)
            z_panel(1)
            z_panel(2)
            final_panel(0)
            z_panel(3)
            final_panel(1)
            final_panel(2)
            final_panel(3)

    nc.compile()
    return nc


def _get_nc():
    if "nc" not in _CACHE:
        _CACHE["nc"] = _build()
    return _CACHE["nc"]


def _prep_weights(Wpo, Wse, bse, coeff, Wag):
    f8 = ml_dtypes.float8_e4m3

    def pack8(W):
        WT = np.ascontiguousarray(np.asarray(W, np.float32).T * WS)
        return np.ascontiguousarray(
            WT.reshape(NQ, 2, P, DE).transpose(0, 2, 1, 3)).astype(f8)

    wpo8 = pack8(Wpo)
    wse8 = pack8(Wse)
    wag = np.ascontiguousarray(
        np.asarray(Wag, np.float32).T).astype(ml_dtypes.bfloat16)
    bseb = np.ascontiguousarray(
        (np.asarray(bse, np.float32) / 6.0 + 0.5).reshape(NE, P).T)
    coeffp = np.ascontiguousarray(
        np.asarray(coeff, np.float32).reshape(NE, P, DEGREE).transpose(1, 0, 2))
    return wpo8, wse8, wag, bseb, coeffp


def kernel(xq, xc, mask, Wpo, Wse, bse, coeff, Wag, _trace=False):
    nc = _get_nc()
    xq = np.ascontiguousarray(xq, np.float32)
    xc = np.ascontiguousarray(xc, np.float32)
    mask = np.ascontiguousarray(mask, np.int32)
    wpo8, wse8, wag, bseb, coeffp = _prep_weights(Wpo, Wse, bse, coeff, Wag)
    maskf = mask.astype(np.float32)
    in_maps = []
    for c in range(N_CORES):
        b, j = c // 2, c % 2
        mh = np.ascontiguousarray(
            maskf[b, j * NSH:(j + 1) * NSH].reshape(NP * NSL, P).T)
        mf = np.ascontiguousarray(maskf[b].reshape(N // P, P).T)
        in_maps.append({
            "xc": np.ascontiguousarray(xc[b, j * NSH:(j + 1) * NSH]),
            "xq": np.ascontiguousarray(xq[b, j * TSH:(j + 1) * TSH]),
            "maskh": mh,
            "maskf": mf,
            "wpo8": wpo8,
            "wse8": wse8,
            "wag": wag,
            "bseb": bseb,
            "coeff": coeffp,
        })
    res = run_bass_kernel_spmd(nc, in_maps, list(range(N_CORES)), trace=_trace)
    out = np.empty((B, T, DIM), np.float32)
    for c in range(N_CORES):
        b, j = c // 2, c % 2
        out[b, j * TSH:(j + 1) * TSH] = res.results[c]["out"]
    if _trace:
        _CACHE["last_result"] = res
    return out
